# revision 4
# baseline (speedup 1.0000x reference)
"""Trainium2 Bass kernel for CollectAttention (PSA 'collect') gather.

out[n, i*W + j, h, w] = x[n, (i-h+H-1)*(2W-1) + (j-w+W-1), h, w]

with N=2, H=W=64, C=(2H-1)*(2W-1)=16129.

Viewing x as [N, A=127, B=127, H, W], the op is the separable diagonal
gather out[n,i,j,h,w] = x[n, i-h+63, j-w+63, h, w].

Strategy (8 NeuronCores), v2 — slab loads:
  - Shard over (n, i-block): core c handles n = c//4 and output rows
    i in [16*(c%4), 16*(c%4)+16).
  - Host feeds each core the layout xs[a_rev, h, b, w] (a-axis sliced
    to the 79-wide window the core needs and REVERSED so all device
    AP steps are non-negative).  In that layout the (a, h) channel
    slab x[a, :, h, :] is 127*64 elems = 32.5KB CONTIGUOUS, so the
    (a,h)-diagonal gather becomes one big DMA descriptor per slab
    instead of the per-(b,w-window) 64-256B descriptors of v1 (which
    all paid the <512B 2x RMW bus penalty).  1024 slabs * 32.5KB =
    33.3MB loaded per core at full bus efficiency, vs v1's 20.6MB at
    half efficiency (41.3MB effective) plus 130k-packet overhead.
  - SBUF partition p = s*32 + h2 in round r holds BOTH h-parity slabs
    (a = i+63-h for h = 2*h2 and 2*h2+1) of output row i: i_loc =
    4r + 3 - s (the s-reversal keeps DRAM steps positive; the host
    un-permutes).  So each partition owns exactly the data of output
    channels (i_loc, :) at rows 2*h2, 2*h2+1.
  - Skew (per r, q=h-parity), one DVE copy over 128 partitions:
      R[p][j*128 + q*64 + w'] = D[p][q*8128 + (j+w')*64 + (63-w')]
    resolves the (b, w) diagonal in the free dim (b = j+w at w=63-w',
    i.e. the stored w axis is reversed; the host flips it back).
  - Store (per r, k): R[p][j*128 + (q,w')] -> out channel
    ((4r + s_phys)*64 + j) rows (2h2, 2h2+1): 512B contiguous HBM
    runs at full bus efficiency.
  - Partitions [0,64) (k=0) are served by the 8 even SDMA engines via
    nc.sync's queue and [64,128) (k=1) by the odd ones via nc.scalar,
    driving disjoint engine sets concurrently.
"""

import numpy as np

N, H, W = 2, 64, 64
R = 2 * H - 1            # 127
C = R * R                # 16129
AWIN = 79                # a-window per core: 16 + 63
NCORES = 8
SLAB = R * W             # 8128 elems: one (a,h) channel slab [b, w]
FDp = 2 * SLAB           # 16256 free elems per partition in a D tile
RFp = 2 * H * W          # 8192 free elems per partition in an R tile
NROUND = 4

_cached = {}


def _build_program():
    import concourse.bass as bass
    import concourse.bacc as bacc
    import concourse.mybir as mybir
    import concourse.tile as tile

    nc = bacc.Bacc(
        "TRN2",
        target_bir_lowering=False,
        debug=False,
        num_devices=NCORES,
    )
    xs = nc.dram_tensor("xs", [AWIN * H * SLAB], mybir.dt.float32, kind="ExternalInput")
    out = nc.dram_tensor("out", [16 * W * H * W], mybir.dt.float32, kind="ExternalOutput")

    f32 = mybir.dt.float32
    with tile.TileContext(nc) as tc:
        with (
            tc.tile_pool(name="dpool", bufs=2) as dpool,
            tc.tile_pool(name="rpool", bufs=2) as rpool,
        ):
            dt = {}
            rt = {}

            def emit_load(r):
                d = dpool.tile([128, FDp], f32, tag="d", name=f"d{r}")
                dt[r] = d
                for k in range(2):
                    eng = nc.sync if k == 0 else nc.scalar
                    for sg in range(2):
                        for q in range(2):
                            # slab index = (12-4r+s_phys+2*h2+q)*64 + 2*h2+q
                            #   with s_phys = 2k+sg
                            base = ((12 - 4 * r + 2 * k + sg) * 64 + 65 * q) * SLAB
                            src = bass.AP(
                                xs,
                                base,
                                [[130 * SLAB, 32], [1, SLAB]],
                            )
                            dst = bass.AP(
                                d.tensor,
                                d.offset + (64 * k + 32 * sg) * FDp + q * SLAB,
                                [[FDp, 32], [1, SLAB]],
                            )
                            eng.dma_start(out=dst, in_=src)

            def emit_skew(r):
                rtile = rpool.tile([128, RFp], f32, tag="r", name=f"r{r}")
                rt[r] = rtile
                d = dt[r]
                for q in range(2):
                    src = bass.AP(
                        d.tensor,
                        d.offset + q * SLAB + 63,
                        [[FDp, 128], [64, 64], [63, 64]],
                    )
                    dst = bass.AP(
                        rtile.tensor,
                        rtile.offset + q * 64,
                        [[RFp, 128], [128, 64], [1, 64]],
                    )
                    nc.vector.tensor_copy(out=dst, in_=src)

            def emit_store(r):
                rtile = rt[r]
                for k in range(2):
                    eng = nc.sync if k == 0 else nc.scalar
                    for sg in range(2):
                        # s_phys = 2k + sg; src: 32 partitions x RFp dense
                        # = one contiguous 262144-elem region.
                        src = bass.AP(
                            rtile.tensor,
                            rtile.offset + (64 * k + 32 * sg) * RFp,
                            [[RFp, 32], [128, 64], [1, 128]],
                        )
                        dst = bass.AP(
                            out,
                            (4 * r + 2 * k + sg) * 64 * 4096,
                            [[128, 32], [4096, 64], [1, 128]],
                        )
                        eng.dma_start(out=dst, in_=src)

            # Software pipeline: stores lag one round so they never reach
            # a DMA queue head before their producer skew has finished
            # (head-of-line blocking stalls the queue's SDMA engines).
            emit_load(0)
            for r in range(NROUND):
                if r + 1 < NROUND:
                    emit_load(r + 1)
                emit_skew(r)
                if r >= 1:
                    emit_store(r - 1)
            emit_store(NROUND - 1)

    nc.compile()
    return nc


def _get_program():
    if "nc" not in _cached:
        _cached["nc"] = _build_program()
    return _cached["nc"]


def shard_input(x: np.ndarray) -> list[dict[str, np.ndarray]]:
    # Per n: [a, b, h, w] -> reverse a -> [a_rev, h, b, w], contiguous.
    xt = {}
    for n in range(N):
        xt[n] = np.ascontiguousarray(
            x[n].reshape(R, R, H, W)[::-1].transpose(0, 2, 1, 3)
        )
    in_maps = []
    for c in range(NCORES):
        n, iblk = c // 4, c % 4
        i0 = 16 * iblk
        # local a_rev = 78 - (a - i0); a in [i0, i0+79) -> global
        # a_rev_g = 126 - a in [48-i0, 127-i0): a contiguous slice.
        xs = xt[n][48 - i0 : 127 - i0]
        in_maps.append({"xs": xs.reshape(-1)})
    return in_maps


def assemble_output(results: list[dict[str, np.ndarray]]) -> np.ndarray:
    out = np.empty((N, H * W, H, W), dtype=np.float32)
    for c in range(NCORES):
        n, iblk = c // 4, c % 4
        # buf[r, s_phys, j, h, w']: i_loc = 4r + 3 - s_phys, w = 63 - w'
        buf = results[c]["out"].reshape(NROUND, 4, W, H, W)
        buf = buf[:, ::-1, :, :, ::-1].reshape(16 * W, H, W)
        out[n, iblk * 1024 : (iblk + 1) * 1024] = buf
    return out


def kernel(x: np.ndarray) -> np.ndarray:
    from concourse.bass_utils import run_bass_kernel_spmd

    x = np.asarray(x, dtype=np.float32)
    assert x.shape == (N, C, H, W), x.shape
    nc = _get_program()
    in_maps = shard_input(x)
    res = run_bass_kernel_spmd(nc, in_maps, list(range(NCORES)))
    return assemble_output(res.results)
